# revision 5
# baseline (speedup 1.0000x reference)
"""GQA causal attention layer (QKV proj + NeoX RoPE + softmax attention + o_proj)
for Trainium2, tensor-parallel over heads across 8 NeuronCores.

Problem shapes (hardcoded): B=1, S=2048, HID=2048, NH=32, NKV=8, HD=64.
Per core c: 4 query heads (4c..4c+3) + 1 kv head (c).

Dataflow (per core, everything "transposed" = feature-on-partition):
  xT [HID, S] (host-transposed input)
  qkvT = w_stat.T @ x:   w_stat cols = [q(256) | v(64) | k(64)] so that
     tile0 = qT heads(0,1) [128,S], tile1 = qT heads(2,3), tile2 = [vT;kT]
  RoPE on qT/kT via  out = qT*C + swap32(qT)*S  (swap via SBUF-SBUF DMA)
  scores ST[k,q] = kT.T @ qT   (K=64 contraction), causal-trimmed, diag mask
  P = exp(0.125*ST)  (no max subtraction; scores are O(10) so exp is safe)
  PV: out_augT = [v | ones].T @ P  -> rows 0-63 attnT, rows 64-127 row-sums
  normalize: attnT *= 1/sums  (ACT reciprocal + DMA partition shift)
  y_partial = attnT.T @ w_o_rows   (row-parallel o_proj)
Host sums the 8 partial outputs.

All matmuls run as float32r (TF32-like, 1 cycle/row at N>=256, ~1.6e-4 rel err).
"""

import numpy as np

import concourse.bass as bass
import concourse.mybir as mybir
import concourse.tile as tile
from concourse import bacc
from concourse import bass_utils
from concourse.masks import make_identity

B, S, HID = 1, 2048, 2048
NH, NKV, HD = 32, 8, 64
NCORES = 8
HPC = NH // NCORES          # 4 query heads per core
ROPE_BASE = 10000.0
SCALE = 1.0 / np.sqrt(HD)   # 0.125
NEG = -1e9

F32 = mybir.dt.float32
F32R = mybir.dt.float32r

KT = S // 128               # 16 k-tiles (of hidden dim in phase 1 / seq in phase 2)
QCHUNK = 1024               # attention q-chunk (2 chunks)
NQC = S // QCHUNK


def _chunks(total, step=512):
    out = []
    o = 0
    while o < total:
        out.append((o, min(step, total - o)))
        o += step
    return out


def build_kernel():
    nc = bacc.Bacc("TRN2", target_bir_lowering=False, debug=False,
                   num_devices=NCORES)

    xT = nc.dram_tensor("xT", [HID, S], F32, kind="ExternalInput").ap()
    w_stat = nc.dram_tensor("w_stat", [HID, 384], F32, kind="ExternalInput").ap()
    w_o = nc.dram_tensor("w_o", [256, HID], F32, kind="ExternalInput").ap()
    Cr = nc.dram_tensor("C", [128, S], F32, kind="ExternalInput").ap()
    Sr = nc.dram_tensor("Sn", [128, S], F32, kind="ExternalInput").ap()
    maskneg = nc.dram_tensor("maskneg", [128, 128], F32, kind="ExternalInput").ap()
    ones64 = nc.dram_tensor("ones64", [128, 64], F32, kind="ExternalInput").ap()
    y = nc.dram_tensor("y", [S, HID], F32, kind="ExternalOutput").ap()

    with tile.TileContext(nc) as tc:
        with (
            tc.tile_pool(name="persist", bufs=1) as pers,
            tc.tile_pool(name="vaugp", bufs=1) as vaugp,
        ):
            # ---- persistent tiles ----
            qkv = [pers.tile([128, S], F32, tag=f"qkv{t}", name=f"qkv{t}") for t in range(3)]
            qr = [pers.tile([128, S], F32R, tag=f"qr{t}", name=f"qr{t}") for t in range(2)]
            kr = pers.tile([128, S], F32R, tag="kr")      # rows 64:128 = roped kT
            kd0 = pers.tile([64, S], F32R, tag="kd0")     # kT copy at base 0
            outstat = [pers.tile([128, S], F32R, tag=f"os{p}", name=f"os{p}") for p in range(2)]
            wo_sb = [pers.tile([128, HID], F32R, tag=f"wo{p}", name=f"wo{p}") for p in range(2)]
            Ct = pers.tile([128, S], F32, tag="Ct")
            St = pers.tile([128, S], F32, tag="St")
            mneg = pers.tile([128, 128], F32, tag="mneg")
            ident = pers.tile([128, 128], F32, tag="ident")
            vaug = [vaugp.tile([128, 128], F32R, tag=f"va{i}", name=f"va{i}") for i in range(KT)]

            nc.sync.dma_start(Ct, Cr)
            nc.sync.dma_start(St, Sr)
            nc.sync.dma_start(mneg, maskneg)
            make_identity(nc, ident)
            for p in range(2):
                nc.sync.dma_start(wo_sb[p], w_o[128 * p:128 * (p + 1), :].bitcast(F32R))

            # ================= Phase 1: qkvT = w_stat.T @ x =================
            with (
                tc.tile_pool(name="xp", bufs=20) as xp,
                tc.tile_pool(name="wp", bufs=KT) as wp,
                tc.tile_pool(name="qkvps", bufs=6, space="PSUM") as qkvps,
            ):
                wt = []
                for k in range(KT):
                    w = wp.tile([128, 384], F32R, tag="w")
                    nc.sync.dma_start(w, w_stat[128 * k:128 * (k + 1), :].bitcast(F32R))
                    wt.append(w)
                for mc in range(4):           # m-chunks of 512
                    m0 = 512 * mc
                    ps = [qkvps.tile([128, 512], F32, tag="qkvps", name=f"qkvps{mc}_{n}") for n in range(3)]
                    for k in range(KT):
                        xt = xp.tile([128, 512], F32R, tag="x")
                        nc.sync.dma_start(
                            xt, xT[128 * k:128 * (k + 1), m0:m0 + 512].bitcast(F32R))
                        for n in range(3):
                            nc.tensor.matmul(
                                ps[n], wt[k][:, 128 * n:128 * (n + 1)], xt,
                                start=(k == 0), stop=(k == KT - 1))
                    for n in range(3):
                        nc.vector.tensor_copy(qkv[n][:, m0:m0 + 512], ps[n])

            # ============ Phase 1.5: RoPE, kT dup, v transpose ============
            with (
                tc.tile_pool(name="swp", bufs=3) as swp,
                tc.tile_pool(name="rtmp", bufs=2) as rtmp,
                tc.tile_pool(name="trps", bufs=4, space="PSUM") as trps,
            ):
                # rope for q tiles 0,1 (full 128 rows) and k (rows 64:128 of qkv[2])
                for t in range(3):
                    src = qkv[t] if t < 2 else qkv[2]
                    r0, r1 = (0, 128) if t < 2 else (64, 128)
                    sw = swp.tile([128, S], F32, tag="sw")
                    for g in range(r0 // 32, r1 // 32, 2):
                        nc.sync.dma_start(sw[32 * g:32 * g + 32, :],
                                          src[32 * g + 32:32 * g + 64, :])
                        nc.sync.dma_start(sw[32 * g + 32:32 * g + 64, :],
                                          src[32 * g:32 * g + 32, :])
                    t1 = rtmp.tile([128, S], F32, tag="t1")
                    t2 = rtmp.tile([128, S], F32, tag="t2")
                    nc.vector.tensor_mul(t1[r0:r1, :], src[r0:r1, :], Ct[r0:r1, :])
                    nc.vector.tensor_mul(t2[r0:r1, :], sw[r0:r1, :], St[r0:r1, :])
                    dst = qr[t] if t < 2 else kr
                    nc.vector.tensor_add(dst[r0:r1, :], t1[r0:r1, :], t2[r0:r1, :])
                # kT duplicate at base 0 (for even heads)
                nc.sync.dma_start(kd0, kr[64:128, :])
                # v transposes: vT rows 0:64 of qkv[2] -> v_aug[:, 0:64]
                for i in range(KT):
                    tp = trps.tile([128, 64], F32, tag="tr")
                    nc.tensor.transpose(
                        tp, qkv[2][0:64, 128 * i:128 * (i + 1)], ident[0:64, 0:64])
                    nc.vector.tensor_copy(vaug[i][:, 0:64], tp)
                    nc.sync.dma_start(vaug[i][:, 64:128], ones64.bitcast(F32R))

            # ================= Phase 2: attention =================
            with (
                tc.tile_pool(name="pp", bufs=3) as pp,
                tc.tile_pool(name="recp", bufs=2) as recp,
                tc.tile_pool(name="rec0p", bufs=2) as rec0p,
                tc.tile_pool(name="otmp", bufs=2) as otmp,
                tc.tile_pool(name="stps", bufs=2, space="PSUM") as stps,
                tc.tile_pool(name="pvps", bufs=2, space="PSUM") as pvps,
            ):
                for h in range(HPC):
                    p, half = h // 2, h % 2
                    qrow = 64 * half
                    if half == 0:
                        ksrc, kb = kd0, 0
                    else:
                        ksrc, kb = kr, 64
                    for j in range(NQC):
                        ilast = 8 * (j + 1) - 1
                        pv = pvps.tile([128, QCHUNK], F32, tag="pv")
                        for i in range(8 * (j + 1)):
                            qstart = max(QCHUNK * j, 128 * i)
                            qlen = QCHUNK * (j + 1) - qstart
                            st = stps.tile([128, QCHUNK], F32, tag="st")
                            for (c0, cl) in _chunks(qlen):
                                nc.tensor.matmul(
                                    st[:, c0:c0 + cl],
                                    ksrc[kb:kb + 64, 128 * i:128 * (i + 1)],
                                    qr[p][qrow:qrow + 64,
                                          qstart + c0:qstart + c0 + cl],
                                    start=True, stop=True)
                            if 128 * i >= QCHUNK * j:
                                nc.vector.tensor_add(
                                    st[:, 0:128], st[:, 0:128], mneg)
                            pt = pp.tile([128, QCHUNK], F32R, tag="p")
                            nc.scalar.activation(
                                pt[:, 0:qlen], st[:, 0:qlen],
                                mybir.ActivationFunctionType.Exp, scale=SCALE)
                            for (c0, cl) in _chunks(qlen):
                                nc.tensor.matmul(
                                    pv[:, qstart - QCHUNK * j + c0:
                                       qstart - QCHUNK * j + c0 + cl],
                                    vaug[i], pt[:, c0:c0 + cl],
                                    start=(i == 0), stop=(i == ilast))
                        # normalize + evict
                        rec = recp.tile([128, QCHUNK], F32, tag="rec")
                        nc.vector.tensor_copy(rec[64:128, :], pv[64:128, :])
                        sums0 = rec0p.tile([64, QCHUNK], F32, tag="sums0")
                        nc.sync.dma_start(sums0, rec[64:128, :])
                        rec0 = rec0p.tile([64, QCHUNK], F32, tag="rec0")
                        nc.vector.reciprocal_approx_fast(rec0, sums0)
                        if half == 0:
                            nc.vector.tensor_mul(
                                outstat[p][0:64, QCHUNK * j:QCHUNK * (j + 1)],
                                pv[0:64, :], rec0)
                        else:
                            ot = otmp.tile([64, QCHUNK], F32R, tag="ot")
                            nc.vector.tensor_mul(ot, pv[0:64, :], rec0)
                            nc.sync.dma_start(
                                outstat[p][64:128, QCHUNK * j:QCHUNK * (j + 1)], ot)

            # ================= Phase 3: o_proj =================
            with (
                tc.tile_pool(name="yp", bufs=4) as yp,
                tc.tile_pool(name="ops", bufs=4, space="PSUM") as ops,
            ):
                for m in range(KT):
                    for nh in range(2):
                        n0 = 1024 * nh
                        ps = ops.tile([128, 1024], F32, tag="o")
                        for p in range(2):
                            for (c0, cl) in _chunks(1024):
                                nc.tensor.matmul(
                                    ps[:, c0:c0 + cl],
                                    outstat[p][:, 128 * m:128 * (m + 1)],
                                    wo_sb[p][:, n0 + c0:n0 + c0 + cl],
                                    start=(p == 0), stop=(p == 1))
                        ysb = yp.tile([128, 1024], F32, tag="y")
                        nc.vector.tensor_copy(ysb, ps)
                        nc.sync.dma_start(y[128 * m:128 * (m + 1), n0:n0 + 1024], ysb)

    nc.compile()
    return nc


def make_host_inputs(x, w_qkv, w_o):
    """Host-side prep: transpose x, per-core weight slices, rope tables."""
    x = np.asarray(x, dtype=np.float32)
    w_qkv = np.asarray(w_qkv, dtype=np.float32)
    w_o = np.asarray(w_o, dtype=np.float32)
    xT = np.ascontiguousarray(x.reshape(S, HID).T)

    inv_freq = 1.0 / (ROPE_BASE ** (np.arange(0, HD, 2, dtype=np.float32) / HD))
    t = np.arange(S, dtype=np.float32)
    freqs = np.outer(t, inv_freq)                     # [S, 32]
    cosT = np.cos(freqs).T.astype(np.float32)         # [32, S]
    sinT = np.sin(freqs).T.astype(np.float32)
    C = np.tile(cosT, (4, 1))                         # [128, S]
    Sn = np.tile(np.concatenate([-sinT, sinT], 0), (2, 1))

    r = np.arange(128)
    maskneg = np.where(r[None, :] < r[:, None], np.float32(NEG),
                       np.float32(0.0)).astype(np.float32)
    ones64 = np.ones((128, 64), dtype=np.float32)

    in_maps = []
    for c in range(NCORES):
        qcols = np.arange(4 * c * HD, 4 * (c + 1) * HD)
        vcols = NH * HD + NKV * HD + np.arange(c * HD, (c + 1) * HD)
        kcols = NH * HD + np.arange(c * HD, (c + 1) * HD)
        w_stat = np.ascontiguousarray(
            np.concatenate([w_qkv[:, qcols], w_qkv[:, vcols], w_qkv[:, kcols]],
                           axis=1))
        w_o_c = np.ascontiguousarray(w_o[256 * c:256 * (c + 1), :])
        in_maps.append({
            "xT": xT, "w_stat": w_stat, "w_o": w_o_c,
            "C": C, "Sn": Sn, "maskneg": maskneg, "ones64": ones64,
        })
    return in_maps


_NC_CACHE = {}


def get_nc():
    if "nc" not in _NC_CACHE:
        _NC_CACHE["nc"] = build_kernel()
    return _NC_CACHE["nc"]


def kernel(x, w_qkv, w_o):
    nc = get_nc()
    in_maps = make_host_inputs(x, w_qkv, w_o)
    res = bass_utils.run_bass_kernel_spmd(nc, in_maps,
                                          core_ids=list(range(NCORES)))
    out = np.zeros((S, HID), dtype=np.float32)
    for c in range(NCORES):
        out += res.results[c]["y"]
    return out.reshape(B, S, HID)


# revision 11
# speedup vs baseline: 27.8876x; 27.8876x over previous
"""GQA causal attention layer (QKV proj + NeoX RoPE + softmax attention + o_proj)
for Trainium2, tensor-parallel over heads across 8 NeuronCores.

Problem shapes (hardcoded): B=1, S=2048, HID=2048, NH=32, NKV=8, HD=64.
Per core c: 4 query heads (4c..4c+3) + 1 kv head (c).

Dataflow (per core, everything "transposed" = feature-on-partition):
  xT [HID, S] (host-transposed input)
  qkvT = w_stat.T @ x, w_stat cols = [q(256) | v(64) | k(64)]:
     tile0 = qT heads(0,1) [128,S], tile1 = qT heads(2,3), tile2 = [vT;kT]
  RoPE on qT/kT via  out = qT*C + swap32(qT)*S (swap via SBUF-SBUF DMA),
  folded into the phase-1 m-chunk loop for overlap; v transposed via PE.
  scores ST[k,q] = kT.T @ qT (K=64), causal-trimmed, diag mask, j-outer loop
  P = exp(0.125*ST)  (no max subtraction; scores are O(10) so exp is safe)
  PV: [v | ones*64].T @ P -> rows 0-63 attnT, rows 64-127 row-sums (replicated)
  normalize: attnT *= 1/sums (fast-reciprocal + DMA partition shift)
  o_proj TRANSPOSED: yT[n,m] = sum_d' w_o[d',n] * attnT[d',m]  (w_o stationary)
Host transposes yT back and sums the 8 partial outputs.

All matmuls run as float32r (TF32-like, 1 cycle/row at N>=256, ~2e-4 rel err).
"""

import numpy as np

import concourse.bass as bass
import concourse.mybir as mybir
import concourse.tile as tile
from concourse import bacc
from concourse import bass_utils
from concourse.masks import make_identity

B, S, HID = 1, 2048, 2048
NH, NKV, HD = 32, 8, 64
NCORES = 8
HPC = NH // NCORES          # 4 query heads per core
ROPE_BASE = 10000.0
SCALE = 1.0 / np.sqrt(HD)   # 0.125
NEG = -1e9

F32 = mybir.dt.float32
F32R = mybir.dt.float32r

KT = S // 128               # 16 tiles of 128
MC = 1024                   # phase-1 m-chunk
NMC = S // MC
QCHUNK = 1024               # attention q-chunk
NQC = S // QCHUNK


def _chunks(total, step=512):
    out = []
    o = 0
    while o < total:
        out.append((o, min(step, total - o)))
        o += step
    return out


def build_kernel(passes=1):
    nc = bacc.Bacc("TRN2", target_bir_lowering=False, debug=False,
                   num_devices=NCORES)

    xT = nc.dram_tensor("xT", [HID, S], F32, kind="ExternalInput").ap()
    w_stat = nc.dram_tensor("w_stat", [HID, 384], F32, kind="ExternalInput").ap()
    w_o = nc.dram_tensor("w_o", [256, HID], F32, kind="ExternalInput").ap()
    Cr = nc.dram_tensor("C", [128, S], F32, kind="ExternalInput").ap()
    Sr = nc.dram_tensor("Sn", [128, S], F32, kind="ExternalInput").ap()
    maskneg = nc.dram_tensor("maskneg", [128, 128], F32, kind="ExternalInput").ap()
    ones64 = nc.dram_tensor("ones64", [128, 64], F32, kind="ExternalInput").ap()
    yT = nc.dram_tensor("yT", [HID, S], F32, kind="ExternalOutput").ap()

    with tile.TileContext(nc) as tc:
      for _pass in range(passes):
        with (
            tc.tile_pool(name="persist", bufs=1, side=None) as pers,
            tc.tile_pool(name="vaugp", bufs=1) as vaugp,
        ):
            # ---- persistent tiles (live across phases) ----
            qr = [pers.tile([128, S], F32R, tag=f"qr{t}", name=f"qr{t}")
                  for t in range(2)]
            # kr rows 64:128 = roped kT (base 64); rows 0:64 = DMA duplicate
            kr = pers.tile([128, S], F32R, tag="kr")
            outstat = [pers.tile([128, S], F32R, tag=f"os{p}", name=f"os{p}")
                       for p in range(2)]
            wo_sb = [pers.tile([128, HID], F32R, tag=f"wo{p}", name=f"wo{p}")
                     for p in range(2)]
            mneg = pers.tile([128, 128], F32, tag="mneg")
            ident = pers.tile([128, 128], F32, tag="ident")
            vaug = [vaugp.tile([128, 128], F32R, tag=f"va{i}", name=f"va{i}")
                    for i in range(KT)]

            nc.scalar.dma_start(mneg, maskneg)
            for i in range(KT):
                nc.scalar.dma_start(vaug[i][:, 64:128], ones64.bitcast(F32R))
            make_identity(nc, ident)
            for p in range(2):
                nc.scalar.dma_start(wo_sb[p],
                                     w_o[128 * p:128 * (p + 1), :].bitcast(F32R))

            # ===== Phase 1: qkvT = w_stat.T @ x, + RoPE + v, per m-chunk =====
            with (
                tc.tile_pool(name="ph1", bufs=1) as ph1,
                tc.tile_pool(name="xp", bufs=6) as xp,
                tc.tile_pool(name="wp", bufs=KT) as wp,
                tc.tile_pool(name="swp", bufs=2) as swp,
                tc.tile_pool(name="rtmp", bufs=1) as rtmp,
                tc.tile_pool(name="qkvps", bufs=3, space="PSUM") as qkvps,
                tc.tile_pool(name="trps", bufs=2, space="PSUM") as trps,
            ):
                qkv = [ph1.tile([128, S], F32, tag=f"qkv{t}", name=f"qkv{t}")
                       for t in range(3)]
                Ct = ph1.tile([128, S], F32, tag="Ct")
                St = ph1.tile([128, S], F32, tag="St")
                nc.scalar.dma_start(Ct, Cr)
                nc.scalar.dma_start(St, Sr)
                wt = []
                for k in range(KT):
                    w = wp.tile([128, 384], F32R, tag="w")
                    nc.scalar.dma_start(
                        w, w_stat[128 * k:128 * (k + 1), :].bitcast(F32R))
                    wt.append(w)
                for mc in range(NMC):
                    m0 = MC * mc
                    ps = [qkvps.tile([128, MC], F32, tag="qkvps",
                                     name=f"qkvps{mc}_{n}") for n in range(3)]
                    for k in range(KT):
                        xt = xp.tile([128, MC], F32R, tag="x")
                        nc.sync.dma_start(
                            xt,
                            xT[128 * k:128 * (k + 1), m0:m0 + MC].bitcast(F32R))
                        for n in range(3):
                            for (c0, cl) in _chunks(MC):
                                nc.tensor.matmul(
                                    ps[n][:, c0:c0 + cl],
                                    wt[k][:, 128 * n:128 * (n + 1)],
                                    xt[:, c0:c0 + cl],
                                    start=(k == 0), stop=(k == KT - 1))
                    for n in range(3):
                        nc.vector.tensor_copy(qkv[n][:, m0:m0 + MC], ps[n])

                    # --- RoPE on this m-chunk ---
                    for t in range(3):
                        src = qkv[t] if t < 2 else qkv[2]
                        r0, r1 = (0, 128) if t < 2 else (64, 128)
                        sw = swp.tile([128, MC], F32, tag=f"sw{t}",
                                      name=f"sw{t}_{mc}")
                        for g in range(r0 // 32, r1 // 32, 2):
                            nc.gpsimd.dma_start(
                                sw[32 * g:32 * g + 32, :],
                                src[32 * g + 32:32 * g + 64, m0:m0 + MC])
                            nc.gpsimd.dma_start(
                                sw[32 * g + 32:32 * g + 64, :],
                                src[32 * g:32 * g + 32, m0:m0 + MC])
                        t1 = rtmp.tile([128, MC], F32, tag="t1")
                        t2 = rtmp.tile([128, MC], F32, tag="t2")
                        nc.vector.tensor_mul(t1[r0:r1, :],
                                             src[r0:r1, m0:m0 + MC],
                                             Ct[r0:r1, m0:m0 + MC])
                        nc.vector.tensor_mul(t2[r0:r1, :], sw[r0:r1, :],
                                             St[r0:r1, m0:m0 + MC])
                        dst = qr[t] if t < 2 else kr
                        nc.vector.tensor_add(dst[r0:r1, m0:m0 + MC],
                                             t1[r0:r1, :], t2[r0:r1, :])
                    # kT duplicate at base 0 (for even heads)
                    nc.gpsimd.dma_start(kr[0:64, m0:m0 + MC],
                                        kr[64:128, m0:m0 + MC])
                    # v transposes for the k-position tiles in this chunk
                    for i in range(8 * mc, 8 * (mc + 1)):
                        tp = trps.tile([128, 64], F32, tag="tr",
                                       name=f"tr{i}")
                        nc.tensor.transpose(
                            tp, qkv[2][0:64, 128 * i:128 * (i + 1)],
                            ident[0:64, 0:64])
                        nc.vector.tensor_copy(vaug[i][:, 0:64], tp)

            # ========== Phase 2+3: attention (j outer) + o_proj ==========
            with (
                tc.tile_pool(name="pp", bufs=3) as pp,
                tc.tile_pool(name="recp", bufs=2) as recp,
                tc.tile_pool(name="rec0p", bufs=2) as rec0p,
                tc.tile_pool(name="otmp", bufs=2) as otmp,
                tc.tile_pool(name="yp", bufs=4) as yp,
                tc.tile_pool(name="stps", bufs=2, space="PSUM") as stps,
                tc.tile_pool(name="pvps", bufs=1, space="PSUM") as pvps,
                tc.tile_pool(name="ops", bufs=2, space="PSUM") as ops,
            ):
                for j in range(NQC):
                    for h in range(HPC):
                        p, half = h // 2, h % 2
                        qrow = 64 * half
                        kb = 0 if half == 0 else 64
                        ilast = 8 * (j + 1) - 1
                        pv = pvps.tile([128, QCHUNK], F32, tag="pv",
                                       name=f"pv{j}_{h}")
                        for i in range(8 * (j + 1)):
                            qstart = max(QCHUNK * j, 128 * i)
                            qlen = QCHUNK * (j + 1) - qstart
                            st = stps.tile([128, QCHUNK], F32, tag="st",
                                           name=f"st{j}_{h}_{i}")
                            for (c0, cl) in _chunks(qlen):
                                nc.tensor.matmul(
                                    st[:, c0:c0 + cl],
                                    kr[kb:kb + 64, 128 * i:128 * (i + 1)],
                                    qr[p][qrow:qrow + 64,
                                          qstart + c0:qstart + c0 + cl],
                                    start=True, stop=True)
                            if 128 * i >= QCHUNK * j:
                                nc.vector.tensor_add(
                                    st[:, 0:128], st[:, 0:128], mneg)
                            pt = pp.tile([128, QCHUNK], F32R, tag="p",
                                         name=f"pt{j}_{h}_{i}")
                            nc.scalar.activation(
                                pt[:, 0:qlen], st[:, 0:qlen],
                                mybir.ActivationFunctionType.Exp, scale=SCALE)
                            for (c0, cl) in _chunks(qlen):
                                nc.tensor.matmul(
                                    pv[:, qstart - QCHUNK * j + c0:
                                       qstart - QCHUNK * j + c0 + cl],
                                    vaug[i], pt[:, c0:c0 + cl],
                                    start=(i == 0), stop=(i == ilast))
                        # normalize + evict into outstat
                        rec = recp.tile([128, QCHUNK], F32, tag="rec")
                        nc.vector.tensor_copy(rec[64:128, :], pv[64:128, :])
                        sums0 = rec0p.tile([64, QCHUNK], F32, tag="sums0")
                        nc.gpsimd.dma_start(sums0, rec[64:128, :])
                        rec0 = rec0p.tile([64, QCHUNK], F32, tag="rec0")
                        nc.vector.reciprocal_approx_fast(rec0, sums0)
                        if half == 0:
                            nc.vector.tensor_mul(
                                outstat[p][0:64, QCHUNK * j:QCHUNK * (j + 1)],
                                pv[0:64, :], rec0)
                        else:
                            ot = otmp.tile([64, QCHUNK], F32R, tag="ot")
                            nc.vector.tensor_mul(ot, pv[0:64, :], rec0)
                            nc.gpsimd.dma_start(
                                outstat[p][64:128,
                                           QCHUNK * j:QCHUNK * (j + 1)], ot)

                    # ---- o_proj for the m-columns finished by this j ----
                    # yT[n, m] = sum_p (w_o_p[:, n]).T @ outstat_p[:, m]
                    for nt in range(KT):
                        for mh in range(QCHUNK // 512):
                            mcol = QCHUNK * j + 512 * mh
                            ps = ops.tile([128, 512], F32, tag="o",
                                          name=f"ops{j}_{nt}_{mh}")
                            for p in range(2):
                                nc.tensor.matmul(
                                    ps, wo_sb[p][:, 128 * nt:128 * (nt + 1)],
                                    outstat[p][:, mcol:mcol + 512],
                                    start=(p == 0), stop=(p == 1))
                            ysb = yp.tile([128, 512], F32, tag="y",
                                          name=f"ysb{j}_{nt}_{mh}")
                            nc.vector.tensor_copy(ysb, ps)
                            nc.scalar.dma_start(
                                yT[128 * nt:128 * (nt + 1), mcol:mcol + 512],
                                ysb)

    nc.compile()
    return nc


def make_host_inputs(x, w_qkv, w_o):
    """Host-side prep: transpose x, per-core weight slices, rope tables."""
    x = np.asarray(x, dtype=np.float32)
    w_qkv = np.asarray(w_qkv, dtype=np.float32)
    w_o = np.asarray(w_o, dtype=np.float32)
    xT = np.ascontiguousarray(x.reshape(S, HID).T)

    inv_freq = 1.0 / (ROPE_BASE ** (np.arange(0, HD, 2, dtype=np.float32) / HD))
    t = np.arange(S, dtype=np.float32)
    freqs = np.outer(t, inv_freq)                     # [S, 32]
    cosT = np.cos(freqs).T.astype(np.float32)         # [32, S]
    sinT = np.sin(freqs).T.astype(np.float32)
    C = np.tile(cosT, (4, 1))                         # [128, S]
    Sn = np.tile(np.concatenate([-sinT, sinT], 0), (2, 1))

    r = np.arange(128)
    maskneg = np.where(r[None, :] < r[:, None], np.float32(NEG),
                       np.float32(0.0)).astype(np.float32)
    ones64 = np.ones((128, 64), dtype=np.float32)

    in_maps = []
    for c in range(NCORES):
        qcols = np.arange(4 * c * HD, 4 * (c + 1) * HD)
        vcols = NH * HD + NKV * HD + np.arange(c * HD, (c + 1) * HD)
        kcols = NH * HD + np.arange(c * HD, (c + 1) * HD)
        w_stat = np.ascontiguousarray(
            np.concatenate([w_qkv[:, qcols], w_qkv[:, vcols], w_qkv[:, kcols]],
                           axis=1))
        w_o_c = np.ascontiguousarray(w_o[256 * c:256 * (c + 1), :])
        in_maps.append({
            "xT": xT, "w_stat": w_stat, "w_o": w_o_c,
            "C": C, "Sn": Sn, "maskneg": maskneg, "ones64": ones64,
        })
    return in_maps


_NC_CACHE = {}


def get_nc():
    if "nc" not in _NC_CACHE:
        _NC_CACHE["nc"] = build_kernel()
    return _NC_CACHE["nc"]


def kernel(x, w_qkv, w_o):
    nc = get_nc()
    in_maps = make_host_inputs(x, w_qkv, w_o)
    res = bass_utils.run_bass_kernel_spmd(nc, in_maps,
                                          core_ids=list(range(NCORES)))
    out = np.zeros((HID, S), dtype=np.float32)
    for c in range(NCORES):
        out += res.results[c]["yT"]
    return np.ascontiguousarray(out.T).reshape(B, S, HID)


# revision 14
# speedup vs baseline: 41.8192x; 1.4996x over previous
"""GQA causal attention layer (QKV proj + NeoX RoPE + softmax attention + o_proj)
for Trainium2, tensor-parallel over heads across 8 NeuronCores.

Problem shapes (hardcoded): B=1, S=2048, HID=2048, NH=32, NKV=8, HD=64.
Per core c: 4 query heads (4c..4c+3) + 1 kv head (c).

Dataflow (per core, everything "transposed" = feature-on-partition):
  xT [HID, S] (host-transposed input)
  qkvT = w_stat.T @ x, w_stat cols = [q(256) | v(64) | k(64)]:
     tile0 = qT heads(0,1) [128,S], tile1 = qT heads(2,3), tile2 = [vT;kT]
  RoPE on qT/kT via  out = qT*C + swap32(qT)*S (swap via SBUF-SBUF DMA),
  folded into the phase-1 m-chunk loop for overlap; v transposed via PE.
  scores ST[k,q] = kT.T @ qT (K=64), causal-trimmed, diag mask, j-outer loop
  P = exp(0.125*ST)  (no max subtraction; scores are O(10) so exp is safe)
  PV: [v | ones*64].T @ P -> rows 0-63 attnT, rows 64-127 row-sums (replicated)
  normalize: attnT *= 1/sums (fast-reciprocal + DMA partition shift)
  o_proj TRANSPOSED: yT[n,m] = sum_d' w_o[d',n] * attnT[d',m]  (w_o stationary)
Host transposes yT back and sums the 8 partial outputs.

All matmuls run as float32r (TF32-like, 1 cycle/row at N>=256, ~2e-4 rel err).
"""

import numpy as np

import concourse.bass as bass
import concourse.mybir as mybir
import concourse.tile as tile
from concourse import bacc
from concourse import bass_utils
from concourse.masks import make_identity

B, S, HID = 1, 2048, 2048
NH, NKV, HD = 32, 8, 64
NCORES = 8
HPC = NH // NCORES          # 4 query heads per core
ROPE_BASE = 10000.0
SCALE = 1.0 / np.sqrt(HD)   # 0.125
NEG = -1e9

F32 = mybir.dt.float32
F32R = mybir.dt.float32r

KT = S // 128               # 16 tiles of 128
MC = 1024                   # phase-1 m-chunk
NMC = S // MC
QCHUNK = 1024               # attention q-chunk
NQC = S // QCHUNK


def _chunks(total, step=512):
    out = []
    o = 0
    while o < total:
        out.append((o, min(step, total - o)))
        o += step
    return out


def build_kernel(passes=1, upto="full"):
    nc = bacc.Bacc("TRN2", target_bir_lowering=False, debug=False,
                   num_devices=NCORES)

    xT = nc.dram_tensor("xT", [HID, S], F32, kind="ExternalInput").ap()
    w_stat = nc.dram_tensor("w_stat", [HID, 384], F32, kind="ExternalInput").ap()
    w_o = nc.dram_tensor("w_o", [256, HID], F32, kind="ExternalInput").ap()
    Cr = nc.dram_tensor("C", [128, S], F32, kind="ExternalInput").ap()
    Sr = nc.dram_tensor("Sn", [128, S], F32, kind="ExternalInput").ap()
    maskneg = nc.dram_tensor("maskneg", [128, 128], F32, kind="ExternalInput").ap()
    ones64 = nc.dram_tensor("ones64", [128, 64], F32, kind="ExternalInput").ap()
    yT = nc.dram_tensor("yT", [HID, S], F32, kind="ExternalOutput").ap()

    with tile.TileContext(nc) as tc:
      for _pass in range(passes):
        with (
            tc.tile_pool(name="persist", bufs=1, side=None) as pers,
            tc.tile_pool(name="vaugp", bufs=1) as vaugp,
        ):
            # ---- persistent tiles (live across phases) ----
            qr = [pers.tile([128, S], F32R, tag=f"qr{t}", name=f"qr{t}")
                  for t in range(2)]
            # kr rows 64:128 = roped kT (base 64); rows 0:64 = DMA duplicate
            kr = pers.tile([128, S], F32R, tag="kr")
            outstat = [pers.tile([128, S], F32R, tag=f"os{p}", name=f"os{p}")
                       for p in range(2)]
            wo_sb = [pers.tile([128, HID], F32R, tag=f"wo{p}", name=f"wo{p}")
                     for p in range(2)]
            mneg = pers.tile([128, 128], F32, tag="mneg")
            ident = pers.tile([128, 128], F32, tag="ident")
            vaug = [vaugp.tile([128, 128], F32R, tag=f"va{i}", name=f"va{i}")
                    for i in range(KT)]

            nc.scalar.dma_start(mneg, maskneg)
            for i in range(KT):
                nc.scalar.dma_start(vaug[i][:, 64:128], ones64.bitcast(F32R))
            make_identity(nc, ident)
            for p in range(2):
                nc.scalar.dma_start(wo_sb[p],
                                     w_o[128 * p:128 * (p + 1), :].bitcast(F32R))

            # ===== Phase 1: qkvT = w_stat.T @ x, + RoPE + v, per m-chunk =====
            if upto == "null":
                pass
            else:
             with (
                tc.tile_pool(name="ph1", bufs=1) as ph1,
                tc.tile_pool(name="xp", bufs=6) as xp,
                tc.tile_pool(name="wp", bufs=KT) as wp,
                tc.tile_pool(name="swp", bufs=2) as swp,
                tc.tile_pool(name="rtmp", bufs=1) as rtmp,
                tc.tile_pool(name="qkvps", bufs=3, space="PSUM") as qkvps,
                tc.tile_pool(name="trps", bufs=2, space="PSUM") as trps,
            ):
                qkv = [ph1.tile([128, S], F32, tag=f"qkv{t}", name=f"qkv{t}")
                       for t in range(3)]
                Ct = ph1.tile([128, S], F32, tag="Ct")
                St = ph1.tile([128, S], F32, tag="St")
                nc.scalar.dma_start(Ct, Cr)
                nc.scalar.dma_start(St, Sr)
                wt = []
                for k in range(KT):
                    w = wp.tile([128, 384], F32R, tag="w")
                    nc.scalar.dma_start(
                        w, w_stat[128 * k:128 * (k + 1), :].bitcast(F32R))
                    wt.append(w)
                for mc in range(NMC):
                    m0 = MC * mc
                    ps = [qkvps.tile([128, MC], F32, tag="qkvps",
                                     name=f"qkvps{mc}_{n}") for n in range(3)]
                    for k in range(KT):
                        xt = xp.tile([128, MC], F32R, tag="x")
                        nc.sync.dma_start(
                            xt,
                            xT[128 * k:128 * (k + 1), m0:m0 + MC].bitcast(F32R))
                        for n in range(3):
                            for (c0, cl) in _chunks(MC):
                                nc.tensor.matmul(
                                    ps[n][:, c0:c0 + cl],
                                    wt[k][:, 128 * n:128 * (n + 1)],
                                    xt[:, c0:c0 + cl],
                                    start=(k == 0), stop=(k == KT - 1))
                    for n in range(3):
                        nc.vector.tensor_copy(qkv[n][:, m0:m0 + MC], ps[n])

                    # --- RoPE on this m-chunk ---
                    for t in range(3):
                        src = qkv[t] if t < 2 else qkv[2]
                        r0, r1 = (0, 128) if t < 2 else (64, 128)
                        sw = swp.tile([128, MC], F32, tag=f"sw{t}",
                                      name=f"sw{t}_{mc}")
                        for g in range(r0 // 32, r1 // 32, 2):
                            nc.gpsimd.dma_start(
                                sw[32 * g:32 * g + 32, :],
                                src[32 * g + 32:32 * g + 64, m0:m0 + MC])
                            nc.gpsimd.dma_start(
                                sw[32 * g + 32:32 * g + 64, :],
                                src[32 * g:32 * g + 32, m0:m0 + MC])
                        t1 = rtmp.tile([128, MC], F32, tag="t1")
                        t2 = rtmp.tile([128, MC], F32, tag="t2")
                        nc.vector.tensor_mul(t1[r0:r1, :],
                                             src[r0:r1, m0:m0 + MC],
                                             Ct[r0:r1, m0:m0 + MC])
                        nc.vector.tensor_mul(t2[r0:r1, :], sw[r0:r1, :],
                                             St[r0:r1, m0:m0 + MC])
                        dst = qr[t] if t < 2 else kr
                        nc.vector.tensor_add(dst[r0:r1, m0:m0 + MC],
                                             t1[r0:r1, :], t2[r0:r1, :])
                    # kT duplicate at base 0 (for even heads)
                    nc.gpsimd.dma_start(kr[0:64, m0:m0 + MC],
                                        kr[64:128, m0:m0 + MC])
                    # v transposes for the k-position tiles in this chunk
                    for i in range(8 * mc, 8 * (mc + 1)):
                        tp = trps.tile([128, 64], F32, tag="tr",
                                       name=f"tr{i}")
                        nc.tensor.transpose(
                            tp, qkv[2][0:64, 128 * i:128 * (i + 1)],
                            ident[0:64, 0:64])
                        nc.vector.tensor_copy(vaug[i][:, 0:64], tp)

            if upto == "null":
                zsb = pers.tile([128, 512], F32, tag="znull")
                nc.gpsimd.memset(zsb, 0.0)
                for nt in range(KT):
                    for mh in range(4):
                        nc.sync.dma_start(
                            yT[128 * nt:128 * (nt + 1),
                               512 * mh:512 * (mh + 1)], zsb)
                continue
            if upto == "phase1":
                zsb1 = pers.tile([128, 512], F32, tag="znull1")
                nc.gpsimd.memset(zsb1, 0.0)
                for nt in range(KT):
                    for mh in range(4):
                        nc.sync.dma_start(
                            yT[128 * nt:128 * (nt + 1),
                               512 * mh:512 * (mh + 1)], zsb1)
                continue
            # ========== Phase 2+3: attention (j outer) + o_proj ==========
            with (
                tc.tile_pool(name="pp", bufs=3) as pp,
                tc.tile_pool(name="recp", bufs=2) as recp,
                tc.tile_pool(name="rec0p", bufs=2) as rec0p,
                tc.tile_pool(name="otmp", bufs=2) as otmp,
                tc.tile_pool(name="yp", bufs=4) as yp,
                tc.tile_pool(name="stps", bufs=2, space="PSUM") as stps,
                tc.tile_pool(name="pvps", bufs=1, space="PSUM") as pvps,
                tc.tile_pool(name="ops", bufs=2, space="PSUM") as ops,
            ):
                for j in range(NQC):
                    for h in range(HPC):
                        p, half = h // 2, h % 2
                        qrow = 64 * half
                        kb = 0 if half == 0 else 64
                        ilast = 8 * (j + 1) - 1
                        pv = pvps.tile([128, QCHUNK], F32, tag="pv",
                                       name=f"pv{j}_{h}")
                        for i in range(8 * (j + 1)):
                            qstart = max(QCHUNK * j, 128 * i)
                            qlen = QCHUNK * (j + 1) - qstart
                            st = stps.tile([128, QCHUNK], F32, tag="st",
                                           name=f"st{j}_{h}_{i}")
                            for (c0, cl) in _chunks(qlen):
                                nc.tensor.matmul(
                                    st[:, c0:c0 + cl],
                                    kr[kb:kb + 64, 128 * i:128 * (i + 1)],
                                    qr[p][qrow:qrow + 64,
                                          qstart + c0:qstart + c0 + cl],
                                    start=True, stop=True)
                            if 128 * i >= QCHUNK * j:
                                nc.vector.tensor_add(
                                    st[:, 0:128], st[:, 0:128], mneg)
                            pt = pp.tile([128, QCHUNK], F32R, tag="p",
                                         name=f"pt{j}_{h}_{i}")
                            nc.scalar.activation(
                                pt[:, 0:qlen], st[:, 0:qlen],
                                mybir.ActivationFunctionType.Exp, scale=SCALE)
                            for (c0, cl) in _chunks(qlen):
                                nc.tensor.matmul(
                                    pv[:, qstart - QCHUNK * j + c0:
                                       qstart - QCHUNK * j + c0 + cl],
                                    vaug[i], pt[:, c0:c0 + cl],
                                    start=(i == 0), stop=(i == ilast))
                        # normalize + evict into outstat
                        rec = recp.tile([128, QCHUNK], F32, tag="rec")
                        nc.vector.tensor_copy(rec[64:128, :], pv[64:128, :])
                        sums0 = rec0p.tile([64, QCHUNK], F32, tag="sums0")
                        nc.gpsimd.dma_start(sums0, rec[64:128, :])
                        rec0 = rec0p.tile([64, QCHUNK], F32, tag="rec0")
                        nc.vector.reciprocal_approx_fast(rec0, sums0)
                        if half == 0:
                            nc.vector.tensor_mul(
                                outstat[p][0:64, QCHUNK * j:QCHUNK * (j + 1)],
                                pv[0:64, :], rec0)
                        else:
                            ot = otmp.tile([64, QCHUNK], F32R, tag="ot")
                            nc.vector.tensor_mul(ot, pv[0:64, :], rec0)
                            nc.gpsimd.dma_start(
                                outstat[p][64:128,
                                           QCHUNK * j:QCHUNK * (j + 1)], ot)

                    # ---- o_proj for the m-columns finished by this j ----
                    # yT[n, m] = sum_p (w_o_p[:, n]).T @ outstat_p[:, m]
                    for nt in range(KT):
                        for mh in range(QCHUNK // 512):
                            mcol = QCHUNK * j + 512 * mh
                            ps = ops.tile([128, 512], F32, tag="o",
                                          name=f"ops{j}_{nt}_{mh}")
                            for p in range(2):
                                nc.tensor.matmul(
                                    ps, wo_sb[p][:, 128 * nt:128 * (nt + 1)],
                                    outstat[p][:, mcol:mcol + 512],
                                    start=(p == 0), stop=(p == 1))
                            ysb = yp.tile([128, 512], F32, tag="y",
                                          name=f"ysb{j}_{nt}_{mh}")
                            if j == 0:
                                nc.vector.tensor_copy(ysb, ps)
                            else:
                                nc.scalar.copy(ysb, ps)
                            nc.scalar.dma_start(
                                yT[128 * nt:128 * (nt + 1), mcol:mcol + 512],
                                ysb)

    nc.compile()
    return nc


def make_host_inputs(x, w_qkv, w_o):
    """Host-side prep: transpose x, per-core weight slices, rope tables."""
    x = np.asarray(x, dtype=np.float32)
    w_qkv = np.asarray(w_qkv, dtype=np.float32)
    w_o = np.asarray(w_o, dtype=np.float32)
    xT = np.ascontiguousarray(x.reshape(S, HID).T)

    inv_freq = 1.0 / (ROPE_BASE ** (np.arange(0, HD, 2, dtype=np.float32) / HD))
    t = np.arange(S, dtype=np.float32)
    freqs = np.outer(t, inv_freq)                     # [S, 32]
    cosT = np.cos(freqs).T.astype(np.float32)         # [32, S]
    sinT = np.sin(freqs).T.astype(np.float32)
    C = np.tile(cosT, (4, 1))                         # [128, S]
    Sn = np.tile(np.concatenate([-sinT, sinT], 0), (2, 1))

    r = np.arange(128)
    maskneg = np.where(r[None, :] < r[:, None], np.float32(NEG),
                       np.float32(0.0)).astype(np.float32)
    ones64 = np.ones((128, 64), dtype=np.float32)

    in_maps = []
    for c in range(NCORES):
        qcols = np.arange(4 * c * HD, 4 * (c + 1) * HD)
        vcols = NH * HD + NKV * HD + np.arange(c * HD, (c + 1) * HD)
        kcols = NH * HD + np.arange(c * HD, (c + 1) * HD)
        w_stat = np.ascontiguousarray(
            np.concatenate([w_qkv[:, qcols], w_qkv[:, vcols], w_qkv[:, kcols]],
                           axis=1))
        w_o_c = np.ascontiguousarray(w_o[256 * c:256 * (c + 1), :])
        in_maps.append({
            "xT": xT, "w_stat": w_stat, "w_o": w_o_c,
            "C": C, "Sn": Sn, "maskneg": maskneg, "ones64": ones64,
        })
    return in_maps


_NC_CACHE = {}


def get_nc():
    if "nc" not in _NC_CACHE:
        _NC_CACHE["nc"] = build_kernel()
    return _NC_CACHE["nc"]


def kernel(x, w_qkv, w_o):
    nc = get_nc()
    in_maps = make_host_inputs(x, w_qkv, w_o)
    res = bass_utils.run_bass_kernel_spmd(nc, in_maps,
                                          core_ids=list(range(NCORES)))
    out = np.zeros((HID, S), dtype=np.float32)
    for c in range(NCORES):
        out += res.results[c]["yT"]
    return np.ascontiguousarray(out.T).reshape(B, S, HID)
